# revision 38
# baseline (speedup 1.0000x reference)
"""Joint attention layer on 8 trn2 NeuronCores (query-sharded, SPMD).

Math (reference):
    Q = img @ Wq.T ; K = text @ Wk.T ; S = Q @ K.T        [N, N]
    attn = softmax(S, axis=1) / sqrt(D)
    out_img = attn @ img ; out_text = attn @ text

Per-core plan (core c owns query rows m in [c*1024, (c+1)*1024)):
    H[j,i]  = sum_d Wq[d,j] Wk[d,i]              (= Wq.T @ Wk, 256x256)
    G[i,m]  = sum_j H[j,i] imgT[j,m]             (absorbs both projections)
    S^T[n,m] = sum_i text[n,i] G[i,m]            (keys on partitions)
    P^T = exp(S^T)  (no max subtraction needed: |S| <~ 55 << 88)
    O[m,:] = sum_n P^T[n,m] * rhs[n,:]           (rhs = [img|text], PSUM accum)
    rowsum[m] = sum_n P^T[n,m]

The device ships UNNORMALIZED O plus the rowsums straight from PSUM to
DRAM; the host does out = O * (1/sqrt(D)) / rowsum.  This removes the
whole on-device epilogue (transpose matmuls, reciprocal, scale copies)
from the critical path — PSUM banks are released by direct PSUM->DRAM
DMAs, so the m-block boundary and the kernel tail cost only DMA time.

The rowsum is NOT computed on the PE per chunk (a [1,512]-out
ones-matmul still streams all 512 columns, ~30us of PE time total).
Instead the otherwise-idle DVE keeps a running pt_acc[k,m] += P^T per
chunk, and one fp32 ones-matmul per m-block reduces pt_acc over its
128 key lanes.

Precision: S-chain (Wq,Wk,H,imgT,G,textT) in fp16 (values are O(1));
P^T and the O matmul in bf16 (exp values reach ~e^55, beyond fp16
range); all matmul accumulation in fp32 PSUM; pt_acc in fp32 SBUF.

Schedule notes (from perfetto traces):
  - NWARM dummy matmuls on memset data run first so the PE HAM clock
    un-throttles (1.2 -> 2.4 GHz) while input DMAs are in flight.
  - Input bandwidth is DMA-descriptor-count bound (~40-80ns/descriptor,
    fanned out across 16 engines). The host packs img|text into one
    [N, 2D] bf16 tensor (1KB descriptors, one dma_start per key chunk)
    and Wq/Wk into one [128, 4*256] tile (2KB descriptors).
  - DMA issue order interleaves textT column-groups with the rhs chunk
    tiles so chunk c's operands land in pace with compute.
  - The two m-blocks run as ONE flat software pipeline over (mb, ch)
    pairs with PIPE-chunk lookahead, so m-block 1's first S tiles and
    exps are emitted during m-block 0's last iterations (no warmup
    bubble at the boundary).
  - The last chunk's DVE add is chained into the previous iteration
    (its P^T tile already exists via the lookahead), so at ch==NCH-1
    nothing on the PE waits on the DVE accumulation chain.

No collectives: outputs are disjoint row slabs concatenated on host.
"""

import numpy as np
import ml_dtypes
from contextlib import ExitStack

import concourse.bass as bass
import concourse.tile as tile
from concourse import bacc, mybir
from concourse.bass_utils import run_bass_kernel_spmd

F32 = mybir.dt.float32
F32R = mybir.dt.float32r
F16 = mybir.dt.float16
BF16 = mybir.dt.bfloat16
P = 128          # partitions
D = 256          # hidden dim
N = 8192         # sequence length
N_CORES = 8
SLAB = N // N_CORES          # 1024 query rows per core
MB = 2                       # m-blocks per core
MBS = SLAB // MB             # 512 queries per m-block
NSUB = MBS // P              # 4 psum subtiles per m-block
NCH = N // P                 # 64 key chunks of 128
TTG = 8                      # textT column-group tiles
TTW = N // TTG               # 1024 cols per group
PIPE = 2                     # S-stage lookahead (chunks)
NWARM = 18                   # HAM-prewarm dummy matmuls (>=3.4us of
                             # sustained PE activity so the HAM clock is
                             # guaranteed warm before the real stream)
NORM = 1.0 / 16.0            # 1/sqrt(D)

_CACHE = {}


def _build_nc():
    nc = bacc.Bacc("TRN2", target_bir_lowering=False, debug=False,
                   num_devices=N_CORES)

    rhsp_d = nc.dram_tensor("rhs_bf16", [N, 2 * D], BF16, kind="ExternalInput").ap()
    textT_d = nc.dram_tensor("textT_f16", [D, N], F16, kind="ExternalInput").ap()
    imgT_d = nc.dram_tensor("imgT_f16", [D, SLAB], F16, kind="ExternalInput").ap()
    wqwk_d = nc.dram_tensor("wqwk_f16", [P, 8 * P], F16, kind="ExternalInput").ap()
    # Unnormalized O ships as bf16 (half the evacuation-copy time and DMA
    # bytes; ~0.4% rounding vs the 2e-2 budget). The rowsum divisor stays
    # fp32.
    out_d = nc.dram_tensor("out", [SLAB, 2 * D], BF16, kind="ExternalOutput").ap()
    rs_d = nc.dram_tensor("rowsum", [MB, MBS], F32, kind="ExternalOutput").ap()

    with tile.TileContext(nc) as tc:
        with ExitStack() as ctx:
            const = ctx.enter_context(tc.tile_pool(name="const", bufs=1))

            # Prewarm fodder: memset tile matmul'd NWARM times into a scratch
            # psum bank so the PE HAM un-throttles while input DMAs land.
            warm_sb = const.tile([P, D], BF16, name="warm_sb")
            nc.vector.memset(warm_sb[:], 0.5)

            # wqwk layout: [Wq rows 0:128 | Wq rows 128:256 | Wk 0:128 |
            # Wk 128:256], each 256 wide. All setup inputs go first on the
            # sync HWDGE queue (the scalar queue measured ~7x slower per
            # descriptor, and these gate the H/G matmuls).
            wqwk_sb = const.tile([P, 8 * P], F16, name="wqwk")
            nc.sync.dma_start(wqwk_sb[:], wqwk_d[:, :])
            imgT_sb = [const.tile([P, SLAB], F16, name=f"imgT{t}") for t in range(2)]
            for t in range(2):
                nc.sync.dma_start(imgT_sb[t][:], imgT_d[t * P:(t + 1) * P, :])

            def wq(dt):
                return wqwk_sb[:, dt * 2 * P:(dt + 1) * 2 * P]

            def wk(dt):
                return wqwk_sb[:, (2 + dt) * 2 * P:(3 + dt) * 2 * P]

            # float32r: the rowsum matmul streams at 1 cycle/row (vs 4 for
            # fp32) when the moving dim is >=256; the BIR verifier requires
            # every producer of an f32r matmul operand to write f32r (so
            # memset an f32 tile, then copy-cast — memset itself can't
            # target f32r).
            ones_f32 = const.tile([P, 1], F32, name="ones_f32")
            nc.vector.memset(ones_f32[:], 1.0)
            ones_sb = const.tile([P, 1], F32R, name="ones")
            nc.vector.tensor_copy(ones_sb[:], ones_f32[:])

            # textT groups + rhs chunk tiles, DMA-issue interleaved so chunk
            # c's operands (tt group c//8, rhs[c]) land in pace with compute.
            tt_sb = [[const.tile([P, TTW], F16, name=f"tt{it}_{g}")
                      for g in range(TTG)] for it in range(2)]
            rhs_pool = ctx.enter_context(tc.tile_pool(name="rhs", bufs=NCH))
            rhs_tiles = {}
            for g in range(TTG):
                for it in range(2):
                    nc.sync.dma_start(
                        tt_sb[it][g][:],
                        textT_d[it * P:(it + 1) * P, g * TTW:(g + 1) * TTW])
                for ch in range(g * 8, (g + 1) * 8):
                    rhs = rhs_pool.tile([P, 2 * D], BF16, tag="rhs",
                                        name=f"rhs{ch}")
                    nc.sync.dma_start(rhs[:], rhsp_d[ch * P:(ch + 1) * P, :])
                    rhs_tiles[ch] = rhs

            h_sb = [const.tile([P, D], F16, name=f"h{jt}") for jt in range(2)]
            g_sb = [const.tile([P, SLAB], F16, name=f"g{it}") for it in range(2)]

            # ---- prewarm + setup: H = Wq.T @ Wk ; G = H-chain @ imgT ----
            with tc.tile_pool(name="psetup", bufs=2, space="PSUM") as psetup:
                warm_ps = psetup.tile([P, D], F32, tag="warm", name="warm_ps")
                for w in range(NWARM):
                    nc.tensor.matmul(warm_ps[:], lhsT=warm_sb[:, 0:P],
                                     rhs=warm_sb[:], start=True, stop=True)
                # PSUM->SBUF evacuation copies alternate DVE/Act so the
                # setup chain (H matmul -> copy -> G matmul -> copy) doesn't
                # serialize on one engine.
                for jt in range(2):
                    hp = psetup.tile([P, D], F32, tag="h", name=f"hp{jt}")
                    for dt in range(2):
                        nc.tensor.matmul(hp[:],
                                         lhsT=wq(dt)[:, jt * P:(jt + 1) * P],
                                         rhs=wk(dt)[:],
                                         start=(dt == 0), stop=(dt == 1))
                    if jt == 0:
                        nc.vector.tensor_copy(h_sb[jt][:], hp[:])
                    else:
                        nc.scalar.activation(h_sb[jt][:], hp[:],
                                             mybir.ActivationFunctionType.Copy)
                for it in range(2):
                    for hh in range(2):
                        gp = psetup.tile([P, MBS], F32, tag="g", name=f"gp{it}_{hh}")
                        for jt in range(2):
                            nc.tensor.matmul(
                                gp[:],
                                lhsT=h_sb[jt][:, it * P:(it + 1) * P],
                                rhs=imgT_sb[jt][:, hh * MBS:(hh + 1) * MBS],
                                start=(jt == 0), stop=(jt == 1))
                        dst = g_sb[it][:, hh * MBS:(hh + 1) * MBS]
                        if hh == 0:
                            nc.vector.tensor_copy(dst, gp[:])
                        else:
                            nc.scalar.activation(dst, gp[:],
                                                 mybir.ActivationFunctionType.Copy)

            # ---- main pools ----
            o_pool = ctx.enter_context(tc.tile_pool(name="opool", bufs=4, space="PSUM"))
            s_pool = ctx.enter_context(tc.tile_pool(name="spool", bufs=PIPE + 1, space="PSUM"))
            rs_pool = ctx.enter_context(tc.tile_pool(name="rspool", bufs=1, space="PSUM"))
            pt_pool = ctx.enter_context(tc.tile_pool(name="pt", bufs=PIPE + 3))
            acc_pool = ctx.enter_context(tc.tile_pool(name="acc", bufs=MB))
            eout_pool = ctx.enter_context(tc.tile_pool(name="eout", bufs=2 * NSUB))
            rsb_pool = ctx.enter_context(tc.tile_pool(name="rsb", bufs=MB))

            def s_mm(mb, ch, it, sp):
                g, coff = divmod(ch, TTW // P)
                coff *= P
                nc.tensor.matmul(
                    sp[:],
                    lhsT=tt_sb[it][g][:, coff:coff + P],
                    rhs=g_sb[it][:, mb * MBS:(mb + 1) * MBS],
                    start=(it == 0), stop=(it == 1))

            def s_act(mb, ch, sp):
                pt = pt_pool.tile([P, MBS], BF16, tag="pt", name=f"pt{mb}_{ch}")
                nc.scalar.activation(pt[:], sp[:],
                                     mybir.ActivationFunctionType.Exp)
                return pt

            seq = [(mb, ch) for mb in range(MB) for ch in range(NCH)]
            pts = {}
            for i in range(PIPE):
                mb, ch = seq[i]
                sp = s_pool.tile([P, MBS], F32, tag="s", name=f"s{mb}_{ch}")
                s_mm(mb, ch, 0, sp)
                s_mm(mb, ch, 1, sp)
                pts[(mb, ch)] = s_act(mb, ch, sp)

            o_ps = {}
            pt_acc = {}
            for i, (mb, ch) in enumerate(seq):
                first, last = (ch == 0), (ch == NCH - 1)
                if first:
                    o_ps[mb] = [o_pool.tile([P, 2 * D], F32, tag="o",
                                            name=f"o{mb}_{s}")
                                for s in range(NSUB)]
                    pt_acc[mb] = acc_pool.tile([P, MBS], F32R, tag="acc",
                                               name=f"ptacc{mb}")

                look = seq[i + PIPE] if i + PIPE < len(seq) else None
                sp_n = None
                if look is not None:
                    sp_n = s_pool.tile([P, MBS], F32, tag="s",
                                       name=f"s{look[0]}_{look[1]}")

                rhs = rhs_tiles[ch]
                pt = pts.pop((mb, ch))

                def o_mm(sub):
                    nc.tensor.matmul(o_ps[mb][sub][:],
                                     lhsT=pt[:, sub * P:(sub + 1) * P],
                                     rhs=rhs[:], start=first, stop=last)

                if last:
                    # The accumulate for this chunk already ran during the
                    # previous iteration (chained add), so the f32r rowsum
                    # matmul can lead the block with no DVE wait.
                    rs_ps = rs_pool.tile([1, MBS], F32, tag="rs",
                                         name=f"rs{mb}")
                    nc.tensor.matmul(rs_ps[:], lhsT=ones_sb[:],
                                     rhs=pt_acc[mb][:], start=True, stop=True)
                    o_mm(0)
                    o_mm(1)
                    o_mm(2)
                    if sp_n is not None:
                        s_mm(*look, 0, sp_n)
                        s_mm(*look, 1, sp_n)
                        pts[look] = s_act(*look, sp_n)
                    o_mm(3)
                    # Evacuate PSUM -> SBUF (DMA can't read PSUM): subtile
                    # copies split across DVE and Act so all four O banks
                    # free within ~1.4us; unnormalized O + rowsum then DMA
                    # out from SBUF (host does the softmax division).
                    rs_sb = rsb_pool.tile([1, MBS], F32, tag="rssb",
                                          name=f"rssb{mb}")
                    osb = [eout_pool.tile([P, 2 * D], BF16, tag="eout",
                                          name=f"eout{mb}_{s}")
                           for s in range(NSUB)]
                    # All store DMAs ride the sync queue: one dma_start per
                    # subtile (its descriptors already fan out across all 16
                    # DMA engines), and the Act engine must stay free of
                    # dma_start issue cost so the next m-block's exps resume
                    # immediately.
                    for sub in range(NSUB):
                        if sub % 2 == 0:
                            nc.vector.tensor_copy(osb[sub][:],
                                                  o_ps[mb][sub][:])
                        else:
                            nc.scalar.activation(
                                osb[sub][:], o_ps[mb][sub][:],
                                mybir.ActivationFunctionType.Copy)
                        row0 = mb * MBS + sub * P
                        # Final m-block: Act has no more exps to run, so its
                        # dma_start issue cost is free — use both queues.
                        eng = (nc.scalar if (mb == MB - 1 and sub % 2 == 1)
                               else nc.sync)
                        eng.dma_start(out_d[row0:row0 + P, :], osb[sub][:])
                    nc.vector.tensor_copy(rs_sb[:], rs_ps[:])
                    nc.sync.dma_start(rs_d[mb:mb + 1, :], rs_sb[:])
                else:
                    if sp_n is not None:
                        s_mm(*look, 0, sp_n)
                    o_mm(0)
                    if sp_n is not None:
                        s_mm(*look, 1, sp_n)
                        pts[look] = s_act(*look, sp_n)
                    o_mm(1)
                    o_mm(2)
                    o_mm(3)
                    # DVE running accumulation (replaces per-chunk PE rowsum
                    # matmuls); ch==0 doubles as the pt_acc init. Emitted
                    # after the O MMs so engine-sync ordering never puts the
                    # O stream behind the DVE chain. The last chunk's add is
                    # chained here at ch==NCH-2 (its pt already exists via
                    # the PIPE lookahead) so nothing in the final block
                    # waits on the DVE.
                    if first:
                        nc.vector.tensor_copy(pt_acc[mb][:], pt[:])
                    else:
                        nc.vector.tensor_tensor(pt_acc[mb][:], pt[:],
                                                pt_acc[mb][:],
                                                mybir.AluOpType.add)
                    if ch == NCH - 2:
                        pt_last = pts[(mb, NCH - 1)]
                        nc.vector.tensor_tensor(pt_acc[mb][:], pt_last[:],
                                                pt_acc[mb][:],
                                                mybir.AluOpType.add)

                # Touch the next textT group's tiles 4 chunks before their
                # first real use: the first LDWEIGHTS reading a fresh tile
                # carries its DMA-semaphore wait and can't be hoisted by the
                # PE's load-ahead window, costing ~160ns inline per group
                # (measured 379ns vs 216ns matmuls at every group switch).
                # A 1-column dummy matmul consumes the wait off-path.
                j = i + 4
                if j < len(seq) and seq[j][1] % 8 == 0:
                    g2 = seq[j][1] // 8
                    tp_ = rs_pool.tile([1, 1], F32, tag="rs",
                                       name=f"ttt{seq[j][0]}_{g2}")
                    for it in range(2):
                        nc.tensor.matmul(tp_[:], lhsT=tt_sb[it][g2][:, 0:1],
                                         rhs=g_sb[0][:, 0:1],
                                         start=True, stop=True,
                                         skip_group_check=True)

    nc.compile()
    return nc


def kernel(img, text, Wq, Wk):
    img = np.ascontiguousarray(img, dtype=np.float32)
    text = np.ascontiguousarray(text, dtype=np.float32)

    if "nc" not in _CACHE:
        _CACHE["nc"] = _build_nc()
    nc = _CACHE["nc"]

    rhsp = np.ascontiguousarray(
        np.concatenate([img, text], axis=1).astype(ml_dtypes.bfloat16))
    textT16 = np.ascontiguousarray(text.T.astype(np.float16))
    wq16 = np.asarray(Wq, dtype=np.float16)
    wk16 = np.asarray(Wk, dtype=np.float16)
    # [Wq rows 0:128 | Wq rows 128:256 | Wk rows 0:128 | Wk rows 128:256]
    wqwk = np.ascontiguousarray(np.concatenate(
        [wq16[0:P, :], wq16[P:2 * P, :], wk16[0:P, :], wk16[P:2 * P, :]],
        axis=1))

    in_maps = []
    for c in range(N_CORES):
        in_maps.append({
            "rhs_bf16": rhsp,
            "textT_f16": textT16,
            "imgT_f16": np.ascontiguousarray(
                img[c * SLAB:(c + 1) * SLAB].T.astype(np.float16)),
            "wqwk_f16": wqwk,
        })

    res = run_bass_kernel_spmd(nc, in_maps, core_ids=list(range(N_CORES)),
                               **_CACHE.get("run_kwargs", {}))
    _CACHE["last_results"] = res
    outs = []
    for c in range(N_CORES):
        o = np.asarray(res.results[c]["out"]).astype(np.float32)
        rs = np.asarray(res.results[c]["rowsum"], dtype=np.float32)
        scale = NORM / rs.reshape(SLAB)          # [1024]
        outs.append(o * scale[:, None])
    out = np.concatenate(outs, axis=0)
    return np.ascontiguousarray(out[:, :D]), np.ascontiguousarray(out[:, D:])


if __name__ == "__main__":
    rng = np.random.default_rng(0)
    img = rng.standard_normal((N, D), dtype=np.float32)
    text = rng.standard_normal((N, D), dtype=np.float32)
    sc = 1.0 / np.sqrt(D)
    Wq = rng.uniform(-sc, sc, (D, D)).astype(np.float32)
    Wk = rng.uniform(-sc, sc, (D, D)).astype(np.float32)
    oi, ot = kernel(img, text, Wq, Wk)
    print("out_img", oi.shape, oi.dtype, "out_text", ot.shape, ot.dtype)


# revision 39
# speedup vs baseline: 1.0244x; 1.0244x over previous
"""Joint attention layer on 8 trn2 NeuronCores (query-sharded, SPMD).

Math (reference):
    Q = img @ Wq.T ; K = text @ Wk.T ; S = Q @ K.T        [N, N]
    attn = softmax(S, axis=1) / sqrt(D)
    out_img = attn @ img ; out_text = attn @ text

Per-core plan (core c owns query rows m in [c*1024, (c+1)*1024)):
    H[j,i]  = sum_d Wq[d,j] Wk[d,i]              (= Wq.T @ Wk, 256x256)
    G[i,m]  = sum_j H[j,i] imgT[j,m]             (absorbs both projections)
    S^T[n,m] = sum_i text[n,i] G[i,m]            (keys on partitions)
    P^T = exp(S^T)  (no max subtraction needed: |S| <~ 55 << 88)
    O[m,:] = sum_n P^T[n,m] * rhs[n,:]           (rhs = [img|text], PSUM accum)
    rowsum[m] = sum_n P^T[n,m]

The device ships UNNORMALIZED O plus the rowsums straight from PSUM to
DRAM; the host does out = O * (1/sqrt(D)) / rowsum.  This removes the
whole on-device epilogue (transpose matmuls, reciprocal, scale copies)
from the critical path — PSUM banks are released by direct PSUM->DRAM
DMAs, so the m-block boundary and the kernel tail cost only DMA time.

The rowsum is NOT computed on the PE per chunk (a [1,512]-out
ones-matmul still streams all 512 columns, ~30us of PE time total).
Instead the otherwise-idle DVE keeps a running pt_acc[k,m] += P^T per
chunk, and one fp32 ones-matmul per m-block reduces pt_acc over its
128 key lanes.

Precision: S-chain (Wq,Wk,H,imgT,G,textT) in fp16 (values are O(1));
P^T and the O matmul in bf16 (exp values reach ~e^55, beyond fp16
range); all matmul accumulation in fp32 PSUM; pt_acc in fp32 SBUF.

Schedule notes (from perfetto traces):
  - NWARM dummy matmuls on memset data run first so the PE HAM clock
    un-throttles (1.2 -> 2.4 GHz) while input DMAs are in flight.
  - Input bandwidth is DMA-descriptor-count bound (~40-80ns/descriptor,
    fanned out across 16 engines). The host packs img|text into one
    [N, 2D] bf16 tensor (1KB descriptors, one dma_start per key chunk)
    and Wq/Wk into one [128, 4*256] tile (2KB descriptors).
  - DMA issue order interleaves textT column-groups with the rhs chunk
    tiles so chunk c's operands land in pace with compute.
  - The two m-blocks run as ONE flat software pipeline over (mb, ch)
    pairs with PIPE-chunk lookahead, so m-block 1's first S tiles and
    exps are emitted during m-block 0's last iterations (no warmup
    bubble at the boundary).
  - The last chunk's DVE add is chained into the previous iteration
    (its P^T tile already exists via the lookahead), so at ch==NCH-1
    nothing on the PE waits on the DVE accumulation chain.

No collectives: outputs are disjoint row slabs concatenated on host.
"""

import numpy as np
import ml_dtypes
from contextlib import ExitStack

import concourse.bass as bass
import concourse.tile as tile
from concourse import bacc, mybir
from concourse.bass_utils import run_bass_kernel_spmd

F32 = mybir.dt.float32
F32R = mybir.dt.float32r
F16 = mybir.dt.float16
BF16 = mybir.dt.bfloat16
P = 128          # partitions
D = 256          # hidden dim
N = 8192         # sequence length
N_CORES = 8
SLAB = N // N_CORES          # 1024 query rows per core
MB = 2                       # m-blocks per core
MBS = SLAB // MB             # 512 queries per m-block
NSUB = MBS // P              # 4 psum subtiles per m-block
NCH = N // P                 # 64 key chunks of 128
TTG = 8                      # textT column-group tiles
TTW = N // TTG               # 1024 cols per group
PIPE = 2                     # S-stage lookahead (chunks)
NWARM = 18                   # HAM-prewarm dummy matmuls (>=3.4us of
                             # sustained PE activity so the HAM clock is
                             # guaranteed warm before the real stream)
NORM = 1.0 / 16.0            # 1/sqrt(D)

_CACHE = {}


def _build_nc():
    nc = bacc.Bacc("TRN2", target_bir_lowering=False, debug=False,
                   num_devices=N_CORES)

    rhsp_d = nc.dram_tensor("rhs_bf16", [N, 2 * D], BF16, kind="ExternalInput").ap()
    textT_d = nc.dram_tensor("textT_f16", [D, N], F16, kind="ExternalInput").ap()
    imgT_d = nc.dram_tensor("imgT_f16", [D, SLAB], F16, kind="ExternalInput").ap()
    wqwk_d = nc.dram_tensor("wqwk_f16", [P, 8 * P], F16, kind="ExternalInput").ap()
    # Unnormalized O ships as bf16 (half the evacuation-copy time and DMA
    # bytes; ~0.4% rounding vs the 2e-2 budget). The rowsum divisor stays
    # fp32.
    out_d = nc.dram_tensor("out", [SLAB, 2 * D], BF16, kind="ExternalOutput").ap()
    rs_d = nc.dram_tensor("rowsum", [MB, MBS], F32, kind="ExternalOutput").ap()

    with tile.TileContext(nc) as tc:
        with ExitStack() as ctx:
            const = ctx.enter_context(tc.tile_pool(name="const", bufs=1))

            # Prewarm fodder: memset tile matmul'd NWARM times into a scratch
            # psum bank so the PE HAM un-throttles while input DMAs land.
            warm_sb = const.tile([P, D], BF16, name="warm_sb")
            nc.vector.memset(warm_sb[:], 0.5)

            # wqwk layout: [Wq rows 0:128 | Wq rows 128:256 | Wk 0:128 |
            # Wk 128:256], each 256 wide. All setup inputs go first on the
            # sync HWDGE queue (the scalar queue measured ~7x slower per
            # descriptor, and these gate the H/G matmuls).
            wqwk_sb = const.tile([P, 8 * P], F16, name="wqwk")
            nc.sync.dma_start(wqwk_sb[:], wqwk_d[:, :])
            imgT_sb = [const.tile([P, SLAB], F16, name=f"imgT{t}") for t in range(2)]
            for t in range(2):
                nc.sync.dma_start(imgT_sb[t][:], imgT_d[t * P:(t + 1) * P, :])

            def wq(dt):
                return wqwk_sb[:, dt * 2 * P:(dt + 1) * 2 * P]

            def wk(dt):
                return wqwk_sb[:, (2 + dt) * 2 * P:(3 + dt) * 2 * P]

            # float32r: the rowsum matmul streams at 1 cycle/row (vs 4 for
            # fp32) when the moving dim is >=256; the BIR verifier requires
            # every producer of an f32r matmul operand to write f32r (so
            # memset an f32 tile, then copy-cast — memset itself can't
            # target f32r).
            ones_f32 = const.tile([P, 1], F32, name="ones_f32")
            nc.vector.memset(ones_f32[:], 1.0)
            ones_sb = const.tile([P, 1], F32R, name="ones")
            nc.vector.tensor_copy(ones_sb[:], ones_f32[:])

            # textT groups + rhs chunk tiles, DMA-issue interleaved so chunk
            # c's operands (tt group c//8, rhs[c]) land in pace with compute.
            tt_sb = [[const.tile([P, TTW], F16, name=f"tt{it}_{g}")
                      for g in range(TTG)] for it in range(2)]
            rhs_pool = ctx.enter_context(tc.tile_pool(name="rhs", bufs=NCH))
            rhs_tiles = {}
            for g in range(TTG):
                for it in range(2):
                    nc.sync.dma_start(
                        tt_sb[it][g][:],
                        textT_d[it * P:(it + 1) * P, g * TTW:(g + 1) * TTW])
                for ch in range(g * 8, (g + 1) * 8):
                    rhs = rhs_pool.tile([P, 2 * D], BF16, tag="rhs",
                                        name=f"rhs{ch}")
                    nc.sync.dma_start(rhs[:], rhsp_d[ch * P:(ch + 1) * P, :])
                    rhs_tiles[ch] = rhs

            h_sb = [const.tile([P, D], F16, name=f"h{jt}") for jt in range(2)]
            g_sb = [const.tile([P, SLAB], F16, name=f"g{it}") for it in range(2)]

            # ---- prewarm + setup: H = Wq.T @ Wk ; G = H-chain @ imgT ----
            with tc.tile_pool(name="psetup", bufs=2, space="PSUM") as psetup:
                warm_ps = psetup.tile([P, D], F32, tag="warm", name="warm_ps")
                for w in range(NWARM):
                    nc.tensor.matmul(warm_ps[:], lhsT=warm_sb[:, 0:P],
                                     rhs=warm_sb[:], start=True, stop=True)
                # PSUM->SBUF evacuation copies alternate DVE/Act so the
                # setup chain (H matmul -> copy -> G matmul -> copy) doesn't
                # serialize on one engine.
                for jt in range(2):
                    hp = psetup.tile([P, D], F32, tag="h", name=f"hp{jt}")
                    for dt in range(2):
                        nc.tensor.matmul(hp[:],
                                         lhsT=wq(dt)[:, jt * P:(jt + 1) * P],
                                         rhs=wk(dt)[:],
                                         start=(dt == 0), stop=(dt == 1))
                    if jt == 0:
                        nc.vector.tensor_copy(h_sb[jt][:], hp[:])
                    else:
                        nc.scalar.activation(h_sb[jt][:], hp[:],
                                             mybir.ActivationFunctionType.Copy)
                for it in range(2):
                    for hh in range(2):
                        gp = psetup.tile([P, MBS], F32, tag="g", name=f"gp{it}_{hh}")
                        for jt in range(2):
                            nc.tensor.matmul(
                                gp[:],
                                lhsT=h_sb[jt][:, it * P:(it + 1) * P],
                                rhs=imgT_sb[jt][:, hh * MBS:(hh + 1) * MBS],
                                start=(jt == 0), stop=(jt == 1))
                        dst = g_sb[it][:, hh * MBS:(hh + 1) * MBS]
                        if hh == 0:
                            nc.vector.tensor_copy(dst, gp[:])
                        else:
                            nc.scalar.activation(dst, gp[:],
                                                 mybir.ActivationFunctionType.Copy)

            # ---- main pools ----
            o_pool = ctx.enter_context(tc.tile_pool(name="opool", bufs=4, space="PSUM"))
            s_pool = ctx.enter_context(tc.tile_pool(name="spool", bufs=PIPE + 1, space="PSUM"))
            rs_pool = ctx.enter_context(tc.tile_pool(name="rspool", bufs=1, space="PSUM"))
            pt_pool = ctx.enter_context(tc.tile_pool(name="pt", bufs=PIPE + 3))
            acc_pool = ctx.enter_context(tc.tile_pool(name="acc", bufs=MB))
            eout_pool = ctx.enter_context(tc.tile_pool(name="eout", bufs=2 * NSUB))
            rsb_pool = ctx.enter_context(tc.tile_pool(name="rsb", bufs=MB))

            def s_mm(mb, ch, it, sp):
                g, coff = divmod(ch, TTW // P)
                coff *= P
                nc.tensor.matmul(
                    sp[:],
                    lhsT=tt_sb[it][g][:, coff:coff + P],
                    rhs=g_sb[it][:, mb * MBS:(mb + 1) * MBS],
                    start=(it == 0), stop=(it == 1))

            def s_act(mb, ch, sp):
                pt = pt_pool.tile([P, MBS], BF16, tag="pt", name=f"pt{mb}_{ch}")
                nc.scalar.activation(pt[:], sp[:],
                                     mybir.ActivationFunctionType.Exp)
                return pt

            seq = [(mb, ch) for mb in range(MB) for ch in range(NCH)]
            pts = {}
            for i in range(PIPE):
                mb, ch = seq[i]
                sp = s_pool.tile([P, MBS], F32, tag="s", name=f"s{mb}_{ch}")
                s_mm(mb, ch, 0, sp)
                s_mm(mb, ch, 1, sp)
                pts[(mb, ch)] = s_act(mb, ch, sp)

            o_ps = {}
            pt_acc = {}
            for i, (mb, ch) in enumerate(seq):
                first, last = (ch == 0), (ch == NCH - 1)
                if first:
                    o_ps[mb] = [o_pool.tile([P, 2 * D], F32, tag="o",
                                            name=f"o{mb}_{s}")
                                for s in range(NSUB)]
                    pt_acc[mb] = acc_pool.tile([P, MBS], F32R, tag="acc",
                                               name=f"ptacc{mb}")

                look = seq[i + PIPE] if i + PIPE < len(seq) else None
                sp_n = None
                if look is not None:
                    sp_n = s_pool.tile([P, MBS], F32, tag="s",
                                       name=f"s{look[0]}_{look[1]}")

                rhs = rhs_tiles[ch]
                pt = pts.pop((mb, ch))

                def o_mm(sub):
                    nc.tensor.matmul(o_ps[mb][sub][:],
                                     lhsT=pt[:, sub * P:(sub + 1) * P],
                                     rhs=rhs[:], start=first, stop=last)

                if last:
                    # The accumulate for this chunk already ran during the
                    # previous iteration (chained add), so the f32r rowsum
                    # matmul can lead the block with no DVE wait.
                    rs_ps = rs_pool.tile([1, MBS], F32, tag="rs",
                                         name=f"rs{mb}")
                    nc.tensor.matmul(rs_ps[:], lhsT=ones_sb[:],
                                     rhs=pt_acc[mb][:], start=True, stop=True)
                    o_mm(0)
                    o_mm(1)
                    o_mm(2)
                    if sp_n is not None:
                        s_mm(*look, 0, sp_n)
                        s_mm(*look, 1, sp_n)
                        pts[look] = s_act(*look, sp_n)
                    o_mm(3)
                    # Evacuate PSUM -> SBUF (DMA can't read PSUM): subtile
                    # copies split across DVE and Act so all four O banks
                    # free within ~1.4us; unnormalized O + rowsum then DMA
                    # out from SBUF (host does the softmax division).
                    rs_sb = rsb_pool.tile([1, MBS], F32, tag="rssb",
                                          name=f"rssb{mb}")
                    osb = [eout_pool.tile([P, 2 * D], BF16, tag="eout",
                                          name=f"eout{mb}_{s}")
                           for s in range(NSUB)]
                    # All store DMAs ride the sync queue: one dma_start per
                    # subtile (its descriptors already fan out across all 16
                    # DMA engines), and the Act engine must stay free of
                    # dma_start issue cost so the next m-block's exps resume
                    # immediately.
                    for sub in range(NSUB):
                        if sub % 2 == 0:
                            nc.vector.tensor_copy(osb[sub][:],
                                                  o_ps[mb][sub][:])
                        else:
                            nc.scalar.activation(
                                osb[sub][:], o_ps[mb][sub][:],
                                mybir.ActivationFunctionType.Copy)
                        row0 = mb * MBS + sub * P
                        # Final m-block: Act has no more exps to run, so its
                        # dma_start issue cost is free — use both queues.
                        eng = (nc.scalar if (mb == MB - 1 and sub % 2 == 1)
                               else nc.sync)
                        eng.dma_start(out_d[row0:row0 + P, :], osb[sub][:])
                    nc.vector.tensor_copy(rs_sb[:], rs_ps[:])
                    nc.sync.dma_start(rs_d[mb:mb + 1, :], rs_sb[:])
                else:
                    if sp_n is not None:
                        s_mm(*look, 0, sp_n)
                    o_mm(0)
                    if sp_n is not None:
                        s_mm(*look, 1, sp_n)
                        pts[look] = s_act(*look, sp_n)
                    o_mm(1)
                    o_mm(2)
                    o_mm(3)
                    # DVE running accumulation (replaces per-chunk PE rowsum
                    # matmuls); ch==0 doubles as the pt_acc init. Emitted
                    # after the O MMs so engine-sync ordering never puts the
                    # O stream behind the DVE chain. The last chunk's add is
                    # chained here at ch==NCH-2 (its pt already exists via
                    # the PIPE lookahead) so nothing in the final block
                    # waits on the DVE.
                    if first:
                        nc.vector.tensor_copy(pt_acc[mb][:], pt[:])
                    else:
                        nc.vector.tensor_tensor(pt_acc[mb][:], pt[:],
                                                pt_acc[mb][:],
                                                mybir.AluOpType.add)
                    if ch == NCH - 2:
                        pt_last = pts[(mb, NCH - 1)]
                        nc.vector.tensor_tensor(pt_acc[mb][:], pt_last[:],
                                                pt_acc[mb][:],
                                                mybir.AluOpType.add)

    nc.compile()
    return nc


def kernel(img, text, Wq, Wk):
    img = np.ascontiguousarray(img, dtype=np.float32)
    text = np.ascontiguousarray(text, dtype=np.float32)

    if "nc" not in _CACHE:
        _CACHE["nc"] = _build_nc()
    nc = _CACHE["nc"]

    rhsp = np.ascontiguousarray(
        np.concatenate([img, text], axis=1).astype(ml_dtypes.bfloat16))
    textT16 = np.ascontiguousarray(text.T.astype(np.float16))
    wq16 = np.asarray(Wq, dtype=np.float16)
    wk16 = np.asarray(Wk, dtype=np.float16)
    # [Wq rows 0:128 | Wq rows 128:256 | Wk rows 0:128 | Wk rows 128:256]
    wqwk = np.ascontiguousarray(np.concatenate(
        [wq16[0:P, :], wq16[P:2 * P, :], wk16[0:P, :], wk16[P:2 * P, :]],
        axis=1))

    in_maps = []
    for c in range(N_CORES):
        in_maps.append({
            "rhs_bf16": rhsp,
            "textT_f16": textT16,
            "imgT_f16": np.ascontiguousarray(
                img[c * SLAB:(c + 1) * SLAB].T.astype(np.float16)),
            "wqwk_f16": wqwk,
        })

    res = run_bass_kernel_spmd(nc, in_maps, core_ids=list(range(N_CORES)),
                               **_CACHE.get("run_kwargs", {}))
    _CACHE["last_results"] = res
    outs = []
    for c in range(N_CORES):
        o = np.asarray(res.results[c]["out"]).astype(np.float32)
        rs = np.asarray(res.results[c]["rowsum"], dtype=np.float32)
        scale = NORM / rs.reshape(SLAB)          # [1024]
        outs.append(o * scale[:, None])
    out = np.concatenate(outs, axis=0)
    return np.ascontiguousarray(out[:, :D]), np.ascontiguousarray(out[:, D:])


if __name__ == "__main__":
    rng = np.random.default_rng(0)
    img = rng.standard_normal((N, D), dtype=np.float32)
    text = rng.standard_normal((N, D), dtype=np.float32)
    sc = 1.0 / np.sqrt(D)
    Wq = rng.uniform(-sc, sc, (D, D)).astype(np.float32)
    Wk = rng.uniform(-sc, sc, (D, D)).astype(np.float32)
    oi, ot = kernel(img, text, Wq, Wk)
    print("out_img", oi.shape, oi.dtype, "out_text", ot.shape, ot.dtype)
